# revision 17
# baseline (speedup 1.0000x reference)
"""Causal multi-head attention with RoPE on 8 Trainium2 NeuronCores (Bass/Tile).

Problem: B=2, S=2048, E=768, H=12 heads, D=64, full rotary (ROPE_DIM=D),
causal softmax, fused QKV + output projection.

Sharding: 8 cores = 2 batches x 4 head-groups (3 heads each).  Each core:
  - loads xT (its batch, host-transposed [E, S]) and its weight slices,
  - computes qT/kT [D, S] and v [S, D] projections on TensorE (fp32r),
  - applies RoPE: rotate_half via a PE permutation matmul (qrot = P @ q),
    bias folded into fused (q+b)*cos / (qrot+P b)*sin VectorE ops,
  - scores sT = kT.T @ qT in [keys, queries] layout (block-skipping the
    strictly-upper causal blocks), exp on ScalarE, causal edge masked with
    gpsimd affine_select, AV accumulated in PSUM with an ones-column in V
    producing the softmax denominator for free,
  - denominator reciprocal via exp(-ln den) on ScalarE, broadcast with a
    K=1 ones-matmul,
  - output projection with its Wp row-slice -> partial y [S, E], emitted
    per query block so it overlaps the remaining attention work.
Host sums the 4 partials per batch and adds (bp + bv @ Wp) once.
"""
import math

import numpy as np

N_HEADS = 12
ROPE_BASE = 10000.0
B, S, E = 2, 2048, 768
D = 64
HPC = 3            # heads per core
N_CORES = 8
QB = 512           # query block (free dim of score tiles)
KB = 128           # key block (partition dim of score tiles)
NQB = S // QB      # 4
NKB = S // KB      # 16
EK = E // 128      # 6 contraction chunks

_RUNNER = None


# ---------------------------------------------------------------- tile patch
def _patch_tile_drain():
    """This container's walrus caps semaphore waits per instruction ("Too
    many sync wait commands").  Split the TileContext tail-drain waits
    across dedicated SP nops."""
    import concourse.tile as tile
    import concourse.mybir as mybir

    if getattr(tile.TileContext, "_drain_patched", False):
        return

    def _drain_and_barrier(self, tick_clock, wait_clock):
        nc = self.nc
        drain_inst = nc.sync.drain()
        wait_clock.add_sem_waits(
            drain_inst.ins, tile.ScopedClock({None: tick_clock.global_clock})
        )
        si = drain_inst.ins.sync_info
        waits = list(si.on_wait) if si is not None else []
        if len(waits) > 1:
            drain_inst.ins.sync_info.on_wait = waits[:1]
            for w in waits[1:]:
                n = nc.sync.nop(nofuse=True)
                n.ins.sync_info = mybir.SyncInfo(on_wait=[w], on_update=[])
        nc.all_engine_barrier()
        assert self.sems is not None
        popped = nc._tile_sem_poison_stack.pop()
        assert popped is self._sem_poison
        nc.clear_and_free_semaphores(list(self.sems.allocated().values()))
        nc.all_engine_barrier()

    tile.TileContext._drain_and_barrier = _drain_and_barrier
    tile.TileContext._drain_patched = True


MAX_WAITS = 1


def _split_waits(nc, maxw=None):
    """Move excess semaphore waits onto same-engine NoOps inserted just
    before the carrying instruction (walrus per-instruction wait cap)."""
    import concourse.mybir as mybir

    if maxw is None:
        maxw = MAX_WAITS
    k = 0
    for f in nc.m.functions:
        for bb in f.blocks:
            new = []
            for ins in bb.instructions:
                si = ins.sync_info
                if si is not None and len(si.on_wait) > maxw:
                    waits = list(si.on_wait)
                    head, tail = waits[:-maxw], waits[-maxw:]
                    for i in range(0, len(head), maxw):
                        nop = mybir.InstNoOp(
                            name=f"{ins.name}-sw{k}", ins=[], outs=[])
                        k += 1
                        nop.engine = ins.engine
                        nop.sync_info = mybir.SyncInfo(
                            on_wait=head[i:i + maxw], on_update=[])
                        new.append(nop)
                    si.on_wait = tail
                new.append(ins)
            bb.instructions = new


# ---------------------------------------------------------------- device IR
def build_bass(reps=1):
    """reps>1 wraps the whole kernel in an on-device For_i repeat loop --
    used only for timing (slope between rep counts removes dispatch
    overhead)."""
    import contextlib
    import concourse.bass as bass
    import concourse.mybir as mybir
    import concourse.tile as tile

    _patch_tile_drain()
    f32 = mybir.dt.float32
    f32r = mybir.dt.float32r
    Act = mybir.ActivationFunctionType
    Alu = mybir.AluOpType

    nc = bass.Bass(enable_partition_id=False)
    xT = nc.dram_tensor("xT", [E, S], f32r, kind="ExternalInput")
    wq = nc.dram_tensor("wq", [E, HPC * D], f32r, kind="ExternalInput")
    wk = nc.dram_tensor("wk", [E, HPC * D], f32r, kind="ExternalInput")
    wv = nc.dram_tensor("wv", [E, 384], f32r, kind="ExternalInput")
    wp = nc.dram_tensor("wp", [HPC * D, E], f32r, kind="ExternalInput")
    biasesd = nc.dram_tensor("biases", [128, 8], f32, kind="ExternalInput")
    cos2 = nc.dram_tensor("cos2", [128, S], f32, kind="ExternalInput")
    sin2 = nc.dram_tensor("sin2", [128, S], f32, kind="ExternalInput")
    smallsd = nc.dram_tensor("smalls", [128, 192], f32r, kind="ExternalInput")
    smallrd = nc.dram_tensor("smallr", [1, 2496], f32r, kind="ExternalInput")
    y = nc.dram_tensor("y", [S, E], f32, kind="ExternalOutput")

    with tile.TileContext(nc) as tc:
        rep_loop = (
            tc.For_i(0, reps, 1,
                     hint_engines=(mybir.EngineType.PE, mybir.EngineType.DVE,
                                   mybir.EngineType.Activation,
                                   mybir.EngineType.Pool, mybir.EngineType.SP))
            if reps > 1 else contextlib.nullcontext()
        )
        with rep_loop, (
            tc.tile_pool(name="consts", bufs=1)
        ) as consts, tc.tile_pool(name="big", bufs=1) as big:
            # ---- constant loads (few large DMAs; x column-halves last so
            # the first projection matmuls start after ~half the x bytes)
            wq_all = consts.tile([128, EK * HPC * D], f32r, tag="wq_all")
            nc.sync.dma_start(
                out=wq_all.rearrange("p (a m) -> p a m", a=EK),
                in_=wq.rearrange("(a p) m -> p a m", p=128))
            wk_all = consts.tile([128, EK * HPC * D], f32r, tag="wk_all")
            nc.sync.dma_start(
                out=wk_all.rearrange("p (a m) -> p a m", a=EK),
                in_=wk.rearrange("(a p) m -> p a m", p=128))
            wv_all = consts.tile([128, EK * 384], f32r, tag="wv_all")
            nc.sync.dma_start(
                out=wv_all.rearrange("p (a m) -> p a m", a=EK),
                in_=wv.rearrange("(a p) m -> p a m", p=128))
            wq_t = [wq_all[:, e * HPC * D:(e + 1) * HPC * D] for e in range(EK)]
            wk_t = [wk_all[:, e * HPC * D:(e + 1) * HPC * D] for e in range(EK)]
            wv_t = [wv_all[:, e * 384:(e + 1) * 384] for e in range(EK)]
            wp0 = consts.tile([128, E], f32r, tag="wp0")
            nc.sync.dma_start(out=wp0, in_=wp[0:128, :])
            wp1 = consts.tile([64, E], f32r, tag="wp1")
            nc.sync.dma_start(out=wp1, in_=wp[128:192, :])
            smalls_t = consts.tile([128, 192], f32r, tag="smalls")
            nc.sync.dma_start(out=smalls_t, in_=smallsd[:, :])
            p2_t = smalls_t[:, 0:128]
            p1_t = smalls_t[0:64, 128:192]
            biases_t = consts.tile([128, 8], f32, tag="biases")
            nc.sync.dma_start(out=biases_t, in_=biasesd[:, :])
            bias_t = {
                nm: (biases_t[:, i:i + 1], biases_t[0:64, 4 + i:5 + i])
                for i, nm in enumerate(("bq", "bk", "pbq", "pbk"))
            }
            smallr_t = consts.tile([1, 2496], f32r, tag="smallr")
            nc.sync.dma_start(out=smallr_t, in_=smallrd[:, :])
            ones_row = smallr_t[:, 0:S]
            ones64 = smallr_t[:, S:S + 64]
            wv7 = smallr_t[:, S + 64:S + 64 + 384]
            cos_t = consts.tile([128, S], f32, tag="cos")
            nc.sync.dma_start(out=cos_t, in_=cos2[:, :])
            sin_t = consts.tile([128, S], f32, tag="sin")
            nc.sync.dma_start(out=sin_t, in_=sin2[:, :])
            xt_all = big.tile([128, EK * S], f32r, tag="xt_all")
            xt3 = xt_all.rearrange("p (a m) -> p a m", a=EK)
            xs3 = xT.rearrange("(a p) m -> p a m", p=128)
            for half in range(2):
                nc.sync.dma_start(
                    out=xt3[:, :, half * 1024:(half + 1) * 1024],
                    in_=xs3[:, :, half * 1024:(half + 1) * 1024])
            xt = [xt_all[:, e * S:(e + 1) * S] for e in range(EK)]

            # ---- long-lived activations
            qTa = big.tile([128, S], f32r, tag="qTa")
            qTb = big.tile([64, S], f32r, tag="qTb")
            kTa = big.tile([128, S], f32r, tag="kTa")
            kTb = big.tile([64, S], f32r, tag="kTb")
            v_sb = [big.tile([128, 384], f32r, tag=f"v{s}", name=f"v{s}")
                    for s in range(NKB)]
            oTa_q = [big.tile([128, QB], f32r, tag=f"oTa{qb}",
                              name=f"oTa{qb}") for qb in range(NQB)]
            oTb_q = [big.tile([64, QB], f32r, tag=f"oTb{qb}",
                              name=f"oTb{qb}") for qb in range(NQB)]

            # ============================ phase 1: projections + RoPE
            # Interleaved emission: q/k tile-halves alternate with v sblocks
            # so PE (matmuls) and DVE/ACT (rope, copies) overlap.
            with (
                tc.tile_pool(name="psq", bufs=2, space="PSUM") as psq_pool,
                tc.tile_pool(name="psrot", bufs=2, space="PSUM") as rot_pool,
                tc.tile_pool(name="psv", bufs=2, space="PSUM") as psv_pool,
                tc.tile_pool(name="ropetmp", bufs=2) as rtmp,
            ):
                def emit_qk_chunk(w_t, bias, pbias, dst, pmat, mP, moff, half):
                    c0 = half * 1024
                    ps = psq_pool.tile([mP, 1024], f32, tag="psq")
                    for e in range(EK):
                        for n in range(2):
                            nc.tensor.matmul(
                                ps[:, n * 512:(n + 1) * 512],
                                lhsT=w_t[e][:, moff:moff + mP],
                                rhs=xt[e][:, c0 + n * 512:c0 + (n + 1) * 512],
                                start=(e == 0), stop=(e == EK - 1),
                            )
                    # unbiased q -> SBUF via ScalarE (idle in this phase)
                    q_sb = rtmp.tile([mP, 1024], f32r, tag="qsb")
                    nc.scalar.copy(q_sb, ps)
                    # qc = (q + b) * cos   (fused bias+mul on DVE)
                    qc = rtmp.tile([mP, 1024], f32, tag="qc")
                    nc.vector.scalar_tensor_tensor(
                        out=qc, in0=ps, scalar=bias[:mP, :],
                        in1=cos_t[:mP, c0:c0 + 1024],
                        op0=Alu.add, op1=Alu.mult)
                    # qrot = P @ q ;  qs = (qrot + P b) * sin
                    qs = rtmp.tile([mP, 1024], f32, tag="qs")
                    for n in range(2):
                        rot = rot_pool.tile([mP, 512], f32, tag="rot")
                        nc.tensor.matmul(
                            rot, lhsT=pmat[:mP, :mP],
                            rhs=q_sb[:, n * 512:(n + 1) * 512],
                            start=True, stop=True)
                        nc.vector.scalar_tensor_tensor(
                            out=qs[:, n * 512:(n + 1) * 512], in0=rot,
                            scalar=pbias[:mP, :],
                            in1=sin_t[:mP, c0 + n * 512:c0 + (n + 1) * 512],
                            op0=Alu.add, op1=Alu.mult)
                    nc.vector.tensor_add(dst[:, c0:c0 + 1024], qc, qs)

                def emit_v_block(s):
                    ps = psv_pool.tile([128, 384], f32, tag="psv")
                    for e in range(EK):
                        nc.tensor.matmul(
                            ps, lhsT=xt[e][:, s * 128:(s + 1) * 128],
                            rhs=wv_t[e], start=(e == 0), stop=False)
                    nc.tensor.matmul(
                        ps, lhsT=ones_row[:, s * 128:(s + 1) * 128],
                        rhs=wv7, start=False, stop=True)
                    nc.vector.tensor_copy(v_sb[s], ps)

                chunks = []
                for w_t, bnm, dsts in ((wq_t, "bq", (qTa, qTb)),
                                       (wk_t, "bk", (kTa, kTb))):
                    for (mP, moff, dst, pmat, bi) in (
                        (128, 0, dsts[0], p2_t, 0),
                        (64, 128, dsts[1], p1_t, 1),
                    ):
                        for half in range(2):
                            chunks.append((w_t, bias_t[bnm][bi],
                                           bias_t["p" + bnm][bi], dst, pmat,
                                           mP, moff, half))
                vs = iter(range(NKB))
                for i, ch in enumerate(chunks):
                    emit_qk_chunk(*ch)
                    for _ in range(2):
                        s = next(vs, None)
                        if s is not None:
                            emit_v_block(s)
                for s in vs:
                    emit_v_block(s)

            # ============================ phase 2+3: attention + y proj
            def v_lhsT(s, h):
                # head values cols [128h..128h+63] + ones col at 128h+64
                return v_sb[s][:, 128 * h:128 * h + 65]

            with (
                tc.tile_pool(name="ps_s", bufs=2, space="PSUM") as s_pool,
                tc.tile_pool(name="ps_ov", bufs=2, space="PSUM") as ov_pool,
                tc.tile_pool(name="ps_dnb", bufs=1, space="PSUM") as dnb_pool,
                tc.tile_pool(name="ps_y", bufs=1, space="PSUM") as y_pool,
                tc.tile_pool(name="pt", bufs=3) as pt_pool,
                tc.tile_pool(name="eps", bufs=2) as ep_pool,
                tc.tile_pool(name="ysb", bufs=2) as ysb_pool,
            ):
                heads = ((qTa, kTa, 0, oTa_q, 0), (qTa, kTa, 64, oTa_q, 64),
                         (qTb, kTb, 0, oTb_q, 0))
                for qb in range(NQB):
                    for h, (qT, kT, p0, oTq, orow) in enumerate(heads):
                        ov = ov_pool.tile([128, 512], f32, tag="ov")
                        qslice = qT[p0:p0 + 64, qb * 512:(qb + 1) * 512]
                        nkb = 4 * (qb + 1)
                        for kp in range(nkb // 2):
                            s2 = s_pool.tile([128, 1024], f32, tag="s2")
                            pt2 = pt_pool.tile([128, 1024], f32r, tag="pt2")
                            for j in range(2):
                                kb = 2 * kp + j
                                nc.tensor.matmul(
                                    s2[:, j * 512:(j + 1) * 512],
                                    lhsT=kT[p0:p0 + 64,
                                            kb * 128:(kb + 1) * 128],
                                    rhs=qslice, start=True, stop=True)
                            nc.scalar.activation(
                                pt2, s2, Act.Exp, scale=1.0 / math.sqrt(D))
                            for j in range(2):
                                kb = 2 * kp + j
                                if kb >= 4 * qb:  # diagonal: causal mask
                                    nc.gpsimd.affine_select(
                                        out=pt2[:, j * 512:(j + 1) * 512],
                                        in_=pt2[:, j * 512:(j + 1) * 512],
                                        compare_op=Alu.is_ge, fill=0.0,
                                        base=qb * 512 - kb * 128,
                                        channel_multiplier=-1,
                                        pattern=[[1, 512]])
                                nc.tensor.matmul(
                                    ov[0:65, :], lhsT=v_lhsT(kb, h),
                                    rhs=pt2[:, j * 512:(j + 1) * 512],
                                    start=(kb == 0), stop=(kb == nkb - 1))
                        # normalize: 1/den = exp(-ln den) on ScalarE,
                        # broadcast via K=1 ones-matmul, multiply on DVE
                        dl = ep_pool.tile([1, 512], f32, tag="dl")
                        nc.scalar.activation(dl, ov[64:65, :], Act.Ln)
                        rec1 = ep_pool.tile([1, 512], f32r, tag="rec1")
                        nc.scalar.activation(rec1, dl, Act.Exp, scale=-1.0)
                        recb_ps = dnb_pool.tile([64, 512], f32, tag="recb_ps")
                        nc.tensor.matmul(recb_ps, lhsT=ones64, rhs=rec1,
                                         start=True, stop=True)
                        recb = ep_pool.tile([64, 512], f32, tag="recb")
                        nc.vector.tensor_copy(recb, recb_ps)
                        nc.vector.tensor_mul(
                            oTq[qb][orow:orow + 64, :], ov[0:64, :], recb)
                    # ---- y projection for this query block (overlaps the
                    # next query block's attention)
                    for mi in range(4):
                        m = 4 * qb + mi
                        y_sb = ysb_pool.tile([128, E], f32, tag="ysb")
                        for (c0, cn) in ((0, 384), (384, 384)):
                            yp = y_pool.tile([128, 384], f32, tag="yp")
                            nc.tensor.matmul(
                                yp,
                                lhsT=oTa_q[qb][:, mi * 128:(mi + 1) * 128],
                                rhs=wp0[:, c0:c0 + cn],
                                start=True, stop=False)
                            nc.tensor.matmul(
                                yp,
                                lhsT=oTb_q[qb][:, mi * 128:(mi + 1) * 128],
                                rhs=wp1[:, c0:c0 + cn],
                                start=False, stop=True)
                            nc.vector.tensor_copy(y_sb[:, c0:c0 + cn], yp)
                        nc.sync.dma_start(
                            out=y[m * 128:(m + 1) * 128, :], in_=y_sb)

    _split_waits(nc)
    return nc


# ---------------------------------------------------------------- runner
class SpmdRunner:
    """Runs a Bass module on the first `n_cores` jax devices via the axon
    PJRT path (mirrors concourse.bass2jax.run_bass_via_pjrt, minus donation
    so the jitted callable is re-invocable for timing)."""

    def __init__(self, nc, n_cores=N_CORES):
        import jax
        import numpy as _np
        from jax.sharding import Mesh, PartitionSpec
        from jax.experimental.shard_map import shard_map
        import concourse.mybir as mybir
        from concourse.bass2jax import _bass_exec_p, install_neuronx_cc_hook

        install_neuronx_cc_hook()
        self.jax = jax
        self.n_cores = n_cores
        in_names, out_names, out_avals, zero_outs = [], [], [], []
        for alloc in nc.m.functions[0].allocations:
            if not isinstance(alloc, mybir.MemoryLocationSet):
                continue
            name = alloc.memorylocations[0].name
            if alloc.kind == "ExternalInput":
                in_names.append(name)
            elif alloc.kind == "ExternalOutput":
                shape = tuple(alloc.tensor_shape)
                dtype = mybir.dt.np(alloc.dtype)
                out_names.append(name)
                out_avals.append(jax.core.ShapedArray(shape, dtype))
                zero_outs.append(_np.zeros(shape, dtype))
        self.in_names, self.out_names = in_names, out_names
        self.out_avals, self.zero_outs = out_avals, zero_outs
        all_names = in_names + out_names

        def _body(*args):
            return tuple(_bass_exec_p.bind(
                *args,
                out_avals=tuple(out_avals),
                in_names=tuple(all_names),
                out_names=tuple(out_names),
                lowering_input_output_aliases=(),
                sim_require_finite=False,
                sim_require_nnan=False,
                nc=nc,
            ))

        devices = jax.devices()[:n_cores]
        self.mesh = Mesh(np.asarray(devices), ("core",))
        nin = len(in_names) + len(out_names)
        self.fn = jax.jit(
            shard_map(_body, mesh=self.mesh,
                      in_specs=(PartitionSpec("core"),) * nin,
                      out_specs=(PartitionSpec("core"),) * len(out_names),
                      check_rep=False),
            keep_unused=True,
        )
        self._dev_args = None

    def prepare(self, in_maps):
        import jax
        from jax.sharding import NamedSharding, PartitionSpec
        sharding = NamedSharding(self.mesh, PartitionSpec("core"))
        concat = [
            np.concatenate([np.ascontiguousarray(m[name]) for m in in_maps],
                           axis=0)
            for name in self.in_names
        ]
        concat += [
            np.zeros((self.n_cores * z.shape[0], *z.shape[1:]), z.dtype)
            for z in self.zero_outs
        ]
        self._dev_args = [jax.device_put(a, sharding) for a in concat]

    def run(self):
        outs = self.fn(*self._dev_args)
        self.jax.block_until_ready(outs)
        return [
            {name: np.asarray(outs[i]).reshape(
                self.n_cores, *self.out_avals[i].shape)[c]
             for i, name in enumerate(self.out_names)}
            for c in range(self.n_cores)
        ]


# ---------------------------------------------------------------- host side
def _rope_tables():
    inv_freq = 1.0 / (ROPE_BASE ** (np.arange(0, D, 2, dtype=np.float64) / D))
    t = np.arange(S, dtype=np.float64)
    freqs = np.outer(t, inv_freq)                      # [S, 32]
    emb = np.concatenate([freqs, freqs], axis=-1)      # [S, 64]
    cosT = np.cos(emb).T.astype(np.float32)            # [64, S]
    sinT = np.sin(emb).T.astype(np.float32)
    return (np.vstack([cosT, cosT]), np.vstack([sinT, sinT]))  # [128, S]


def _perm_mat():
    P = np.zeros((D, D), np.float32)
    for i in range(32):
        P[i, i + 32] = -1.0
        P[i + 32, i] = 1.0
    return P


def make_in_maps(x, Wq, bq, Wk, bk, Wv, bv, Wp, bp):
    cos2, sin2 = _rope_tables()
    P = _perm_mat()
    P2 = np.zeros((128, 128), np.float32)
    P2[:64, :64] = P
    P2[64:, 64:] = P
    smalls = np.zeros((128, 192), np.float32)
    smalls[:, 0:128] = P2.T
    smalls[0:64, 128:192] = P.T
    smallr = np.zeros((1, 2496), np.float32)
    smallr[0, 0:S] = 1.0            # ones row
    smallr[0, S:S + 64] = 1.0       # ones64
    in_maps = []
    for c in range(N_CORES):
        b, g = c // 4, c % 4
        hs = slice(192 * g, 192 * (g + 1))
        wv_s = np.zeros((E, 384), np.float32)
        wv7 = np.zeros((1, 384), np.float32)
        for h in range(HPC):
            wv_s[:, 128 * h:128 * h + 64] = \
                Wv[:, 192 * g + 64 * h:192 * g + 64 * (h + 1)]
            wv7[0, 128 * h + 64] = 1.0
        smallr_c = smallr.copy()
        smallr_c[0, S + 64:S + 64 + 384] = wv7[0]
        bq_s = bq[hs].astype(np.float32)
        bk_s = bk[hs].astype(np.float32)
        pbq = np.concatenate([P @ bq_s[64 * h:64 * (h + 1)]
                              for h in range(HPC)])
        pbk = np.concatenate([P @ bk_s[64 * h:64 * (h + 1)]
                              for h in range(HPC)])
        biases = np.zeros((128, 8), np.float32)
        for i, vec in enumerate((bq_s, bk_s, pbq, pbk)):
            biases[:, i] = vec[0:128]
            biases[0:64, 4 + i] = vec[128:192]
        in_maps.append({
            "xT": np.ascontiguousarray(x[b].T),
            "wq": np.ascontiguousarray(Wq[:, hs]),
            "wk": np.ascontiguousarray(Wk[:, hs]),
            "wv": wv_s,
            "wp": np.ascontiguousarray(Wp[hs, :]),
            "biases": biases,
            "cos2": cos2, "sin2": sin2,
            "smalls": smalls, "smallr": smallr_c,
        })
    return in_maps


def get_runner():
    global _RUNNER
    if _RUNNER is None:
        nc = build_bass()
        _RUNNER = SpmdRunner(nc, N_CORES)
    return _RUNNER


def assemble(results, Wp, bp, bv):
    y = np.zeros((B, S, E), np.float32)
    for c in range(N_CORES):
        y[c // 4] += results[c]["y"]
    y += (bp + bv @ Wp).astype(np.float32)
    return y


def kernel(x, Wq, bq, Wk, bk, Wv, bv, Wp, bp):
    runner = get_runner()
    runner.prepare(make_in_maps(x, Wq, bq, Wk, bk, Wv, bv, Wp, bp))
    return assemble(runner.run(), Wp, bp, bv)


# revision 18
# speedup vs baseline: 1.2649x; 1.2649x over previous
"""Causal multi-head attention with RoPE on 8 Trainium2 NeuronCores (Bass/Tile).

Problem: B=2, S=2048, E=768, H=12 heads, D=64, full rotary (ROPE_DIM=D),
causal softmax, fused QKV + output projection.

Sharding: 8 cores = 2 batches x 4 head-groups (3 heads each).  Each core:
  - loads xT (its batch, host-transposed [E, S]) and its weight slices,
  - computes qT/kT [D, S] and v [S, D] projections on TensorE (fp32r),
  - applies RoPE: rotate_half via a PE permutation matmul (qrot = P @ q),
    bias folded into fused (q+b)*cos / (qrot+P b)*sin VectorE ops,
  - scores sT = kT.T @ qT in [keys, queries] layout (block-skipping the
    strictly-upper causal blocks), exp on ScalarE, causal edge masked with
    gpsimd affine_select, AV accumulated in PSUM with an ones-column in V
    producing the softmax denominator for free,
  - denominator reciprocal via exp(-ln den) on ScalarE, broadcast with a
    K=1 ones-matmul,
  - output projection with its Wp row-slice -> partial y [S, E], emitted
    per query block so it overlaps the remaining attention work.
Host sums the 4 partials per batch and adds (bp + bv @ Wp) once.
"""
import math

import numpy as np

N_HEADS = 12
ROPE_BASE = 10000.0
B, S, E = 2, 2048, 768
D = 64
HPC = 3            # heads per core
N_CORES = 8
QB = 512           # query block (free dim of score tiles)
KB = 128           # key block (partition dim of score tiles)
NQB = S // QB      # 4
NKB = S // KB      # 16
EK = E // 128      # 6 contraction chunks

_RUNNER = None


# ---------------------------------------------------------------- tile patch
def _patch_tile_drain():
    """This container's walrus caps semaphore waits per instruction ("Too
    many sync wait commands").  Split the TileContext tail-drain waits
    across dedicated SP nops."""
    import concourse.tile as tile
    import concourse.mybir as mybir

    if getattr(tile.TileContext, "_drain_patched", False):
        return

    def _drain_and_barrier(self, tick_clock, wait_clock):
        nc = self.nc
        drain_inst = nc.sync.drain()
        wait_clock.add_sem_waits(
            drain_inst.ins, tile.ScopedClock({None: tick_clock.global_clock})
        )
        si = drain_inst.ins.sync_info
        waits = list(si.on_wait) if si is not None else []
        if len(waits) > 1:
            drain_inst.ins.sync_info.on_wait = waits[:1]
            for w in waits[1:]:
                n = nc.sync.nop(nofuse=True)
                n.ins.sync_info = mybir.SyncInfo(on_wait=[w], on_update=[])
        nc.all_engine_barrier()
        assert self.sems is not None
        popped = nc._tile_sem_poison_stack.pop()
        assert popped is self._sem_poison
        nc.clear_and_free_semaphores(list(self.sems.allocated().values()))
        nc.all_engine_barrier()

    tile.TileContext._drain_and_barrier = _drain_and_barrier
    tile.TileContext._drain_patched = True


MAX_WAITS = 1


def _split_waits(nc, maxw=None):
    """Move excess semaphore waits onto same-engine NoOps inserted just
    before the carrying instruction (walrus per-instruction wait cap)."""
    import concourse.mybir as mybir

    if maxw is None:
        maxw = MAX_WAITS
    k = 0
    for f in nc.m.functions:
        for bb in f.blocks:
            new = []
            for ins in bb.instructions:
                si = ins.sync_info
                if si is not None and len(si.on_wait) > maxw:
                    waits = list(si.on_wait)
                    head, tail = waits[:-maxw], waits[-maxw:]
                    for i in range(0, len(head), maxw):
                        nop = mybir.InstNoOp(
                            name=f"{ins.name}-sw{k}", ins=[], outs=[])
                        k += 1
                        nop.engine = ins.engine
                        nop.sync_info = mybir.SyncInfo(
                            on_wait=head[i:i + maxw], on_update=[])
                        new.append(nop)
                    si.on_wait = tail
                new.append(ins)
            bb.instructions = new


# ---------------------------------------------------------------- device IR
def build_bass(reps=1):
    """reps>1 wraps the whole kernel in an on-device For_i repeat loop --
    used only for timing (slope between rep counts removes dispatch
    overhead)."""
    import contextlib
    import concourse.bass as bass
    import concourse.mybir as mybir
    import concourse.tile as tile

    _patch_tile_drain()
    f32 = mybir.dt.float32
    f32r = mybir.dt.float32r
    Act = mybir.ActivationFunctionType
    Alu = mybir.AluOpType

    nc = bass.Bass(enable_partition_id=False)
    xT = nc.dram_tensor("xT", [E, S], f32r, kind="ExternalInput")
    wq = nc.dram_tensor("wq", [E, HPC * D], f32r, kind="ExternalInput")
    wk = nc.dram_tensor("wk", [E, HPC * D], f32r, kind="ExternalInput")
    wv = nc.dram_tensor("wv", [E, 384], f32r, kind="ExternalInput")
    wp = nc.dram_tensor("wp", [HPC * D, E], f32r, kind="ExternalInput")
    biasesd = nc.dram_tensor("biases", [128, 8], f32, kind="ExternalInput")
    cos2 = nc.dram_tensor("cos2", [128, S], f32, kind="ExternalInput")
    sin2 = nc.dram_tensor("sin2", [128, S], f32, kind="ExternalInput")
    smallsd = nc.dram_tensor("smalls", [128, 192], f32r, kind="ExternalInput")
    smallrd = nc.dram_tensor("smallr", [1, 2496], f32r, kind="ExternalInput")
    y = nc.dram_tensor("y", [S, E], f32, kind="ExternalOutput")

    with tile.TileContext(nc) as tc:
        rep_loop = (
            tc.For_i(0, reps, 1,
                     hint_engines=(mybir.EngineType.PE, mybir.EngineType.DVE,
                                   mybir.EngineType.Activation,
                                   mybir.EngineType.Pool, mybir.EngineType.SP))
            if reps > 1 else contextlib.nullcontext()
        )
        with rep_loop, (
            tc.tile_pool(name="consts", bufs=1)
        ) as consts, tc.tile_pool(name="big", bufs=1) as big:
            # ---- constant loads (few large DMAs; x column-halves last so
            # the first projection matmuls start after ~half the x bytes)
            wq_all = consts.tile([128, EK * HPC * D], f32r, tag="wq_all")
            nc.sync.dma_start(
                out=wq_all.rearrange("p (a m) -> p a m", a=EK),
                in_=wq.rearrange("(a p) m -> p a m", p=128))
            wk_all = consts.tile([128, EK * HPC * D], f32r, tag="wk_all")
            nc.sync.dma_start(
                out=wk_all.rearrange("p (a m) -> p a m", a=EK),
                in_=wk.rearrange("(a p) m -> p a m", p=128))
            wv_all = consts.tile([128, EK * 384], f32r, tag="wv_all")
            nc.sync.dma_start(
                out=wv_all.rearrange("p (a m) -> p a m", a=EK),
                in_=wv.rearrange("(a p) m -> p a m", p=128))
            wq_t = [wq_all[:, e * HPC * D:(e + 1) * HPC * D] for e in range(EK)]
            wk_t = [wk_all[:, e * HPC * D:(e + 1) * HPC * D] for e in range(EK)]
            wv_t = [wv_all[:, e * 384:(e + 1) * 384] for e in range(EK)]
            smalls_t = consts.tile([128, 192], f32r, tag="smalls")
            nc.sync.dma_start(out=smalls_t, in_=smallsd[:, :])
            p2_t = smalls_t[:, 0:128]
            p1_t = smalls_t[0:64, 128:192]
            biases_t = consts.tile([128, 8], f32, tag="biases")
            nc.sync.dma_start(out=biases_t, in_=biasesd[:, :])
            bias_t = {
                nm: (biases_t[:, i:i + 1], biases_t[0:64, 4 + i:5 + i])
                for i, nm in enumerate(("bq", "bk", "pbq", "pbk"))
            }
            smallr_t = consts.tile([1, 2496], f32r, tag="smallr")
            nc.sync.dma_start(out=smallr_t, in_=smallrd[:, :])
            ones_row = smallr_t[:, 0:S]
            ones64 = smallr_t[:, S:S + 64]
            wv7 = smallr_t[:, S + 64:S + 64 + 384]
            xt_all = big.tile([128, EK * S], f32r, tag="xt_all")
            xt3 = xt_all.rearrange("p (a m) -> p a m", a=EK)
            xs3 = xT.rearrange("(a p) m -> p a m", p=128)
            for half in range(2):
                nc.sync.dma_start(
                    out=xt3[:, :, half * 1024:(half + 1) * 1024],
                    in_=xs3[:, :, half * 1024:(half + 1) * 1024])
            xt = [xt_all[:, e * S:(e + 1) * S] for e in range(EK)]
            cos_t = consts.tile([128, S], f32, tag="cos")
            nc.sync.dma_start(out=cos_t, in_=cos2[:, :])
            sin_t = consts.tile([128, S], f32, tag="sin")
            nc.sync.dma_start(out=sin_t, in_=sin2[:, :])
            wp0 = consts.tile([128, E], f32r, tag="wp0")
            nc.sync.dma_start(out=wp0, in_=wp[0:128, :])
            wp1 = consts.tile([64, E], f32r, tag="wp1")
            nc.sync.dma_start(out=wp1, in_=wp[128:192, :])

            # ---- long-lived activations
            qTa = big.tile([128, S], f32r, tag="qTa")
            qTb = big.tile([64, S], f32r, tag="qTb")
            kTa = big.tile([128, S], f32r, tag="kTa")
            kTb = big.tile([64, S], f32r, tag="kTb")
            v_sb = [big.tile([128, 384], f32r, tag=f"v{s}", name=f"v{s}")
                    for s in range(NKB)]
            oTa_q = [big.tile([128, QB], f32r, tag=f"oTa{qb}",
                              name=f"oTa{qb}") for qb in range(NQB)]
            oTb_q = [big.tile([64, QB], f32r, tag=f"oTb{qb}",
                              name=f"oTb{qb}") for qb in range(NQB)]

            # ============================ phase 1: projections + RoPE
            # Interleaved emission: q/k tile-halves alternate with v sblocks
            # so PE (matmuls) and DVE/ACT (rope, copies) overlap.
            with (
                tc.tile_pool(name="psq", bufs=2, space="PSUM") as psq_pool,
                tc.tile_pool(name="psrot", bufs=2, space="PSUM") as rot_pool,
                tc.tile_pool(name="psv", bufs=2, space="PSUM") as psv_pool,
                tc.tile_pool(name="ropetmp", bufs=2) as rtmp,
            ):
                def emit_qk_chunk(w_t, bias, pbias, dst, pmat, mP, moff, half):
                    c0 = half * 1024
                    ps = psq_pool.tile([mP, 1024], f32, tag="psq")
                    for e in range(EK):
                        for n in range(2):
                            nc.tensor.matmul(
                                ps[:, n * 512:(n + 1) * 512],
                                lhsT=w_t[e][:, moff:moff + mP],
                                rhs=xt[e][:, c0 + n * 512:c0 + (n + 1) * 512],
                                start=(e == 0), stop=(e == EK - 1),
                            )
                    # unbiased q -> SBUF via ScalarE (idle in this phase)
                    q_sb = rtmp.tile([mP, 1024], f32r, tag="qsb")
                    nc.scalar.copy(q_sb, ps)
                    # qc = (q + b) * cos   (fused bias+mul on DVE)
                    qc = rtmp.tile([mP, 1024], f32, tag="qc")
                    nc.vector.scalar_tensor_tensor(
                        out=qc, in0=ps, scalar=bias[:mP, :],
                        in1=cos_t[:mP, c0:c0 + 1024],
                        op0=Alu.add, op1=Alu.mult)
                    # qrot = P @ q ;  qs = (qrot + P b) * sin
                    qs = rtmp.tile([mP, 1024], f32, tag="qs")
                    for n in range(2):
                        rot = rot_pool.tile([mP, 512], f32, tag="rot")
                        nc.tensor.matmul(
                            rot, lhsT=pmat[:mP, :mP],
                            rhs=q_sb[:, n * 512:(n + 1) * 512],
                            start=True, stop=True)
                        nc.vector.scalar_tensor_tensor(
                            out=qs[:, n * 512:(n + 1) * 512], in0=rot,
                            scalar=pbias[:mP, :],
                            in1=sin_t[:mP, c0 + n * 512:c0 + (n + 1) * 512],
                            op0=Alu.add, op1=Alu.mult)
                    nc.vector.tensor_add(dst[:, c0:c0 + 1024], qc, qs)

                def emit_v_block(s):
                    ps = psv_pool.tile([128, 384], f32, tag="psv")
                    for e in range(EK):
                        nc.tensor.matmul(
                            ps, lhsT=xt[e][:, s * 128:(s + 1) * 128],
                            rhs=wv_t[e], start=(e == 0), stop=False)
                    nc.tensor.matmul(
                        ps, lhsT=ones_row[:, s * 128:(s + 1) * 128],
                        rhs=wv7, start=False, stop=True)
                    nc.vector.tensor_copy(v_sb[s], ps)

                chunks = []
                for w_t, bnm, dsts in ((wq_t, "bq", (qTa, qTb)),
                                       (wk_t, "bk", (kTa, kTb))):
                    for (mP, moff, dst, pmat, bi) in (
                        (128, 0, dsts[0], p2_t, 0),
                        (64, 128, dsts[1], p1_t, 1),
                    ):
                        for half in range(2):
                            chunks.append((w_t, bias_t[bnm][bi],
                                           bias_t["p" + bnm][bi], dst, pmat,
                                           mP, moff, half))
                vs = iter(range(NKB))
                for i, ch in enumerate(chunks):
                    emit_qk_chunk(*ch)
                    for _ in range(2):
                        s = next(vs, None)
                        if s is not None:
                            emit_v_block(s)
                for s in vs:
                    emit_v_block(s)

            # ============================ phase 2+3: attention + y proj
            def v_lhsT(s, h):
                # head values cols [128h..128h+63] + ones col at 128h+64
                return v_sb[s][:, 128 * h:128 * h + 65]

            with (
                tc.tile_pool(name="ps_s", bufs=2, space="PSUM") as s_pool,
                tc.tile_pool(name="ps_ov", bufs=2, space="PSUM") as ov_pool,
                tc.tile_pool(name="ps_dnb", bufs=1, space="PSUM") as dnb_pool,
                tc.tile_pool(name="ps_y", bufs=1, space="PSUM") as y_pool,
                tc.tile_pool(name="pt", bufs=4) as pt_pool,
                tc.tile_pool(name="eps", bufs=2) as ep_pool,
                tc.tile_pool(name="ysb", bufs=2) as ysb_pool,
            ):
                heads = ((qTa, kTa, 0, oTa_q, 0), (qTa, kTa, 64, oTa_q, 64),
                         (qTb, kTb, 0, oTb_q, 0))
                for qb in range(NQB):
                    for h, (qT, kT, p0, oTq, orow) in enumerate(heads):
                        ov = ov_pool.tile([128, 512], f32, tag="ov")
                        qslice = qT[p0:p0 + 64, qb * 512:(qb + 1) * 512]
                        nkb = 4 * (qb + 1)
                        for kp in range(nkb // 2):
                            s2 = s_pool.tile([128, 1024], f32, tag="s2")
                            pt2 = pt_pool.tile([128, 1024], f32r, tag="pt2")
                            for j in range(2):
                                kb = 2 * kp + j
                                nc.tensor.matmul(
                                    s2[:, j * 512:(j + 1) * 512],
                                    lhsT=kT[p0:p0 + 64,
                                            kb * 128:(kb + 1) * 128],
                                    rhs=qslice, start=True, stop=True)
                            nc.scalar.activation(
                                pt2, s2, Act.Exp, scale=1.0 / math.sqrt(D))
                            for j in range(2):
                                kb = 2 * kp + j
                                if kb >= 4 * qb:  # diagonal: causal mask
                                    nc.gpsimd.affine_select(
                                        out=pt2[:, j * 512:(j + 1) * 512],
                                        in_=pt2[:, j * 512:(j + 1) * 512],
                                        compare_op=Alu.is_ge, fill=0.0,
                                        base=qb * 512 - kb * 128,
                                        channel_multiplier=-1,
                                        pattern=[[1, 512]])
                                nc.tensor.matmul(
                                    ov[0:65, :], lhsT=v_lhsT(kb, h),
                                    rhs=pt2[:, j * 512:(j + 1) * 512],
                                    start=(kb == 0), stop=(kb == nkb - 1))
                        # normalize: 1/den = exp(-ln den) on ScalarE,
                        # broadcast via K=1 ones-matmul, multiply on DVE
                        dl = ep_pool.tile([1, 512], f32, tag="dl")
                        nc.scalar.activation(dl, ov[64:65, :], Act.Ln)
                        rec1 = ep_pool.tile([1, 512], f32r, tag="rec1")
                        nc.scalar.activation(rec1, dl, Act.Exp, scale=-1.0)
                        recb_ps = dnb_pool.tile([64, 512], f32, tag="recb_ps")
                        nc.tensor.matmul(recb_ps, lhsT=ones64, rhs=rec1,
                                         start=True, stop=True)
                        recb = ep_pool.tile([64, 512], f32, tag="recb")
                        nc.vector.tensor_copy(recb, recb_ps)
                        nc.vector.tensor_mul(
                            oTq[qb][orow:orow + 64, :], ov[0:64, :], recb)
                    # ---- y projection for this query block (overlaps the
                    # next query block's attention)
                    for mi in range(4):
                        m = 4 * qb + mi
                        y_sb = ysb_pool.tile([128, E], f32, tag="ysb")
                        for (c0, cn) in ((0, 384), (384, 384)):
                            yp = y_pool.tile([128, 384], f32, tag="yp")
                            nc.tensor.matmul(
                                yp,
                                lhsT=oTa_q[qb][:, mi * 128:(mi + 1) * 128],
                                rhs=wp0[:, c0:c0 + cn],
                                start=True, stop=False)
                            nc.tensor.matmul(
                                yp,
                                lhsT=oTb_q[qb][:, mi * 128:(mi + 1) * 128],
                                rhs=wp1[:, c0:c0 + cn],
                                start=False, stop=True)
                            nc.vector.tensor_copy(y_sb[:, c0:c0 + cn], yp)
                        nc.sync.dma_start(
                            out=y[m * 128:(m + 1) * 128, :], in_=y_sb)

    _split_waits(nc)
    return nc


# ---------------------------------------------------------------- runner
class SpmdRunner:
    """Runs a Bass module on the first `n_cores` jax devices via the axon
    PJRT path (mirrors concourse.bass2jax.run_bass_via_pjrt, minus donation
    so the jitted callable is re-invocable for timing)."""

    def __init__(self, nc, n_cores=N_CORES):
        import jax
        import numpy as _np
        from jax.sharding import Mesh, PartitionSpec
        from jax.experimental.shard_map import shard_map
        import concourse.mybir as mybir
        from concourse.bass2jax import _bass_exec_p, install_neuronx_cc_hook

        install_neuronx_cc_hook()
        self.jax = jax
        self.n_cores = n_cores
        in_names, out_names, out_avals, zero_outs = [], [], [], []
        for alloc in nc.m.functions[0].allocations:
            if not isinstance(alloc, mybir.MemoryLocationSet):
                continue
            name = alloc.memorylocations[0].name
            if alloc.kind == "ExternalInput":
                in_names.append(name)
            elif alloc.kind == "ExternalOutput":
                shape = tuple(alloc.tensor_shape)
                dtype = mybir.dt.np(alloc.dtype)
                out_names.append(name)
                out_avals.append(jax.core.ShapedArray(shape, dtype))
                zero_outs.append(_np.zeros(shape, dtype))
        self.in_names, self.out_names = in_names, out_names
        self.out_avals, self.zero_outs = out_avals, zero_outs
        all_names = in_names + out_names

        def _body(*args):
            return tuple(_bass_exec_p.bind(
                *args,
                out_avals=tuple(out_avals),
                in_names=tuple(all_names),
                out_names=tuple(out_names),
                lowering_input_output_aliases=(),
                sim_require_finite=False,
                sim_require_nnan=False,
                nc=nc,
            ))

        devices = jax.devices()[:n_cores]
        self.mesh = Mesh(np.asarray(devices), ("core",))
        nin = len(in_names) + len(out_names)
        self.fn = jax.jit(
            shard_map(_body, mesh=self.mesh,
                      in_specs=(PartitionSpec("core"),) * nin,
                      out_specs=(PartitionSpec("core"),) * len(out_names),
                      check_rep=False),
            keep_unused=True,
        )
        self._dev_args = None

    def prepare(self, in_maps):
        import jax
        from jax.sharding import NamedSharding, PartitionSpec
        sharding = NamedSharding(self.mesh, PartitionSpec("core"))
        concat = [
            np.concatenate([np.ascontiguousarray(m[name]) for m in in_maps],
                           axis=0)
            for name in self.in_names
        ]
        concat += [
            np.zeros((self.n_cores * z.shape[0], *z.shape[1:]), z.dtype)
            for z in self.zero_outs
        ]
        self._dev_args = [jax.device_put(a, sharding) for a in concat]

    def run(self):
        outs = self.fn(*self._dev_args)
        self.jax.block_until_ready(outs)
        return [
            {name: np.asarray(outs[i]).reshape(
                self.n_cores, *self.out_avals[i].shape)[c]
             for i, name in enumerate(self.out_names)}
            for c in range(self.n_cores)
        ]


# ---------------------------------------------------------------- host side
def _rope_tables():
    inv_freq = 1.0 / (ROPE_BASE ** (np.arange(0, D, 2, dtype=np.float64) / D))
    t = np.arange(S, dtype=np.float64)
    freqs = np.outer(t, inv_freq)                      # [S, 32]
    emb = np.concatenate([freqs, freqs], axis=-1)      # [S, 64]
    cosT = np.cos(emb).T.astype(np.float32)            # [64, S]
    sinT = np.sin(emb).T.astype(np.float32)
    return (np.vstack([cosT, cosT]), np.vstack([sinT, sinT]))  # [128, S]


def _perm_mat():
    P = np.zeros((D, D), np.float32)
    for i in range(32):
        P[i, i + 32] = -1.0
        P[i + 32, i] = 1.0
    return P


def make_in_maps(x, Wq, bq, Wk, bk, Wv, bv, Wp, bp):
    cos2, sin2 = _rope_tables()
    P = _perm_mat()
    P2 = np.zeros((128, 128), np.float32)
    P2[:64, :64] = P
    P2[64:, 64:] = P
    smalls = np.zeros((128, 192), np.float32)
    smalls[:, 0:128] = P2.T
    smalls[0:64, 128:192] = P.T
    smallr = np.zeros((1, 2496), np.float32)
    smallr[0, 0:S] = 1.0            # ones row
    smallr[0, S:S + 64] = 1.0       # ones64
    in_maps = []
    for c in range(N_CORES):
        b, g = c // 4, c % 4
        hs = slice(192 * g, 192 * (g + 1))
        wv_s = np.zeros((E, 384), np.float32)
        wv7 = np.zeros((1, 384), np.float32)
        for h in range(HPC):
            wv_s[:, 128 * h:128 * h + 64] = \
                Wv[:, 192 * g + 64 * h:192 * g + 64 * (h + 1)]
            wv7[0, 128 * h + 64] = 1.0
        smallr_c = smallr.copy()
        smallr_c[0, S + 64:S + 64 + 384] = wv7[0]
        bq_s = bq[hs].astype(np.float32)
        bk_s = bk[hs].astype(np.float32)
        pbq = np.concatenate([P @ bq_s[64 * h:64 * (h + 1)]
                              for h in range(HPC)])
        pbk = np.concatenate([P @ bk_s[64 * h:64 * (h + 1)]
                              for h in range(HPC)])
        biases = np.zeros((128, 8), np.float32)
        for i, vec in enumerate((bq_s, bk_s, pbq, pbk)):
            biases[:, i] = vec[0:128]
            biases[0:64, 4 + i] = vec[128:192]
        in_maps.append({
            "xT": np.ascontiguousarray(x[b].T),
            "wq": np.ascontiguousarray(Wq[:, hs]),
            "wk": np.ascontiguousarray(Wk[:, hs]),
            "wv": wv_s,
            "wp": np.ascontiguousarray(Wp[hs, :]),
            "biases": biases,
            "cos2": cos2, "sin2": sin2,
            "smalls": smalls, "smallr": smallr_c,
        })
    return in_maps


def get_runner():
    global _RUNNER
    if _RUNNER is None:
        nc = build_bass()
        _RUNNER = SpmdRunner(nc, N_CORES)
    return _RUNNER


def assemble(results, Wp, bp, bv):
    y = np.zeros((B, S, E), np.float32)
    for c in range(N_CORES):
        y[c // 4] += results[c]["y"]
    y += (bp + bv @ Wp).astype(np.float32)
    return y


def kernel(x, Wq, bq, Wk, bk, Wv, bv, Wp, bp):
    runner = get_runner()
    runner.prepare(make_in_maps(x, Wq, bq, Wk, bk, Wv, bv, Wp, bp))
    return assemble(runner.run(), Wp, bp, bv)
